# revision 20
# baseline (speedup 1.0000x reference)
"""Trainium2 Bass kernel for nn_Mixer_81733227643296 (segment_reduce).

Computes, for single (B,16,256) and pairwise (B,256,64):
  out = concat([single, mean_up_bcast, mean_down_bcast, pair_up, pair_down], axis=2)
with out shape (B, 16, 896), B = 2048.

Sharding: pure data parallel over the batch across 8 NeuronCores (256 each).

Per-core structure, two independent pipelines:

1) single/means path (2 mega-tiles of 128 batch):
   - load single in batch-partition layout (fully linear DMA, 4KB packets)
   - spin sums via a 3-level DVE add tree along the free dim (exact fp32)
   - store singles straight from the tile to out[:, :, 0:256]
   - store means with a replication DMA: a 0-stride middle AP dim writes each
     batch row 16x to out[:, :, 256:768] (gpsimd/SWDGE supports 0-stride)

2) pairwise path (32 tiles of 8 batch):
   - pairwise tile partition p holds rows (2p, 2p+1) -> 512B DMA runs
   - pair segment sums on TensorE: combined up/down mask (M=32) fp32 matmuls
     accumulated over 2 K-chunks; psum layout (spin*16+elec, batch*64+f)
   - psum -> SBUF copy, then per-spin store DMA to out[:, :, 768:896]

DMA queue spread: loads on the SP HWDGE ring, stores on the ACT HWDGE ring,
means replication on gpsimd SWDGE.
"""

import numpy as np

N_EL = 16
N_UP = 8
N_DOWN = N_EL - N_UP
N_PAIR = N_EL * N_EL  # 240 off-diagonal pairs + 16 self entries
B_FULL = 2048
N_CORES = 8
B_LOCAL = B_FULL // N_CORES  # 256
F_SINGLE = 256
F_PAIR = 64
F_OUT = 3 * F_SINGLE + 2 * F_PAIR  # 896

NB_ITER = 8                    # batch items per pairwise iteration
N_ITERS = B_LOCAL // NB_ITER   # 32
NB_MEGA = 128                  # batch items per single/means mega-tile
N_MEGA = B_LOCAL // NB_MEGA    # 2


def _pairwise_masks(n_el, n_up):
    # identical to the reference construction
    eye = ~np.eye(n_el, dtype=bool)
    ups = np.zeros(n_el, dtype=bool)
    ups[:n_up] = True
    downs = ~ups
    up_rows, down_rows = [], []
    for e in range(n_el):
        m = np.zeros((n_el, n_el), dtype=bool)
        m[e, :] = ups
        mu = m[eye].reshape(-1)
        eu = np.zeros(n_el, dtype=bool)
        eu[e] = e < n_up
        up_rows.append(np.concatenate([mu, eu]))
        m = np.zeros((n_el, n_el), dtype=bool)
        m[e, :] = downs
        md = m[eye].reshape(-1)
        ed = np.zeros(n_el, dtype=bool)
        ed[e] = e >= n_up
        down_rows.append(np.concatenate([md, ed]))
    return np.stack(up_rows), np.stack(down_rows)  # each (n_el, n_el^2)


def _constants():
    mu, md = _pairwise_masks(N_EL, N_UP)
    mc = np.concatenate([mu, md], axis=0).astype(np.float32)  # (32, 256)
    # partition p of the pairwise tile holds rows 2p and 2p+1
    lhsT_pair_a = (mc[:, 0::2] / N_UP).T.copy()   # (128, 32), chunk rows 2p
    lhsT_pair_b = (mc[:, 1::2] / N_UP).T.copy()   # (128, 32), chunk rows 2p+1
    return lhsT_pair_a, lhsT_pair_b


def _build_module():
    import concourse.bass as bass
    import concourse.mybir as mybir
    import concourse.tile as tile
    from concourse import bacc

    f32 = mybir.dt.float32
    nc = bacc.Bacc("TRN2", target_bir_lowering=False, debug=False)

    single_d = nc.dram_tensor(
        "single", [B_LOCAL, N_EL, F_SINGLE], f32, kind="ExternalInput"
    ).ap()
    pair_d = nc.dram_tensor(
        "pairwise", [B_LOCAL, N_PAIR, F_PAIR], f32, kind="ExternalInput"
    ).ap()
    mpa_d = nc.dram_tensor("mask_pair_a", [128, 32], f32, kind="ExternalInput").ap()
    mpb_d = nc.dram_tensor("mask_pair_b", [128, 32], f32, kind="ExternalInput").ap()
    out_d = nc.dram_tensor(
        "out", [B_LOCAL, N_EL, F_OUT], f32, kind="ExternalOutput"
    ).ap()

    with tile.TileContext(nc) as tc:
        with (
            tc.tile_pool(name="consts", bufs=1) as cpool,
            tc.tile_pool(name="sgl", bufs=2) as sgl_pool,
            tc.tile_pool(name="tree", bufs=2) as tree_pool,
            tc.tile_pool(name="sums", bufs=2) as sums_pool,
            tc.tile_pool(name="pw", bufs=16) as pw_pool,
            tc.tile_pool(name="pairc", bufs=8) as pairc_pool,
            tc.tile_pool(name="psum_pair", bufs=8, space="PSUM") as psum_pair_pool,
        ):
            mpa = cpool.tile([128, 32], f32)
            nc.sync.dma_start(out=mpa[:], in_=mpa_d[:])
            mpb = cpool.tile([128, 32], f32)
            nc.sync.dma_start(out=mpb[:], in_=mpb_d[:])

            # ---- single/means path: mega loads up front; the two big
            # singles stores and two means stores are spaced through the pw
            # stream (pw prefetch depth rides through each store burst) ----
            sbs, sums_t = [], []
            for h in range(N_MEGA):
                b0 = h * NB_MEGA
                sb = sgl_pool.tile([128, N_EL * F_SINGLE], f32)
                nc.sync.dma_start(
                    out=sb[:],
                    in_=single_d[b0 : b0 + NB_MEGA].rearrange("b e f -> b (e f)"),
                )
                sbs.append(sb)

            def tree_path(h):
                sv = sbs[h][:].rearrange("b (e f) -> b e f", e=N_EL)
                tA = tree_pool.tile([128, 8 * F_SINGLE], f32)
                tAv = tA[:].rearrange("b (e f) -> b e f", e=8)
                nc.vector.tensor_add(
                    out=tAv[:, 0:4, :], in0=sv[:, 0:4, :], in1=sv[:, 4:8, :]
                )
                nc.vector.tensor_add(
                    out=tAv[:, 4:8, :], in0=sv[:, 8:12, :], in1=sv[:, 12:16, :]
                )
                tB = tree_pool.tile([128, 4 * F_SINGLE], f32)
                tBv = tB[:].rearrange("b (e f) -> b e f", e=4)
                nc.vector.tensor_add(
                    out=tBv[:, 0:2, :], in0=tAv[:, 0:2, :], in1=tAv[:, 2:4, :]
                )
                nc.vector.tensor_add(
                    out=tBv[:, 2:4, :], in0=tAv[:, 4:6, :], in1=tAv[:, 6:8, :]
                )
                sums = sums_pool.tile([128, 2 * F_SINGLE], f32)
                nc.vector.tensor_add(
                    out=sums[:, 0:F_SINGLE], in0=tBv[:, 0, :], in1=tBv[:, 1, :]
                )
                nc.vector.tensor_add(
                    out=sums[:, F_SINGLE:], in0=tBv[:, 2, :], in1=tBv[:, 3, :]
                )
                nc.scalar.mul(sums[:], sums[:], 1.0 / N_UP)
                sums_t.append(sums)

            def store_singles(h):
                b0 = h * NB_MEGA
                sv = sbs[h][:].rearrange("b (e f) -> b e f", e=N_EL)
                nc.gpsimd.dma_start(
                    out=out_d[b0 : b0 + NB_MEGA, :, 0:F_SINGLE], in_=sv
                )

            def store_means(h):
                b0 = h * NB_MEGA
                sums = sums_t[h]
                rep_src = bass.AP(
                    tensor=sums[:].tensor,
                    offset=sums[:].offset,
                    ap=[sums[:].ap[0], [0, N_EL], sums[:].ap[1]],
                )
                nc.gpsimd.dma_start(
                    out=out_d[b0 : b0 + NB_MEGA, :, F_SINGLE : 3 * F_SINGLE],
                    in_=rep_src,
                )

            # ---- pairwise path: 32 tiles of 8 batch, processed in pairs so
            # consecutive matmuls share stationary weights (A,A,B,B order) ----
            for it2 in range(N_ITERS // 2):
                if it2 == 4:
                    store_singles(0)
                if it2 == 6:
                    tree_path(0)
                    store_means(0)
                if it2 == 10:
                    store_singles(1)
                if it2 == 12:
                    tree_path(1)
                    store_means(1)
                pws, ppairs = [], []
                for j in range(2):
                    it = it2 * 2 + j
                    b0 = it * NB_ITER
                    pw = pw_pool.tile([128, NB_ITER * 2 * F_PAIR], f32)
                    src_pair = pair_d[b0 : b0 + NB_ITER].rearrange(
                        "b (p c) f -> p b (c f)", p=128
                    )
                    dst_pair = pw[:].rearrange("p (b cf) -> p b cf", b=NB_ITER)
                    nc.sync.dma_start(out=dst_pair, in_=src_pair)
                    pws.append(pw)
                    ppairs.append(
                        psum_pair_pool.tile(
                            [32, NB_ITER * F_PAIR], f32, name="ppair"
                        )
                    )
                for j in range(2):
                    pwv = pws[j][:].rearrange(
                        "p (b c f) -> p b c f", b=NB_ITER, c=2
                    )
                    nc.tensor.matmul(
                        ppairs[j][:], mpa[:], pwv[:, :, 0, :],
                        start=True, stop=False,
                    )
                for j in range(2):
                    pwv = pws[j][:].rearrange(
                        "p (b c f) -> p b c f", b=NB_ITER, c=2
                    )
                    nc.tensor.matmul(
                        ppairs[j][:], mpb[:], pwv[:, :, 1, :],
                        start=False, stop=True,
                    )
                for j in range(2):
                    it = it2 * 2 + j
                    b0 = it * NB_ITER
                    # pairc free size padded 512->576 so the per-spin store AP
                    # can't greedily merge (i,b) into an illegal partition step
                    pairc = pairc_pool.tile([32, NB_ITER * F_PAIR + F_PAIR], f32)
                    nc.vector.tensor_copy(
                        out=pairc[:, 0 : NB_ITER * F_PAIR], in_=ppairs[j][:]
                    )
                    # pair features: per-spin store, loop order (i, b, f)
                    for s in range(2):
                        c0 = 3 * F_SINGLE + s * F_PAIR
                        src_pc = pairc[
                            s * N_EL : (s + 1) * N_EL, 0 : NB_ITER * F_PAIR
                        ].rearrange("i (b f) -> i b f", b=NB_ITER)
                        dst_pc = out_d[
                            b0 : b0 + NB_ITER, :, c0 : c0 + F_PAIR
                        ].rearrange("b e f -> e b f")
                        nc.scalar.dma_start(out=dst_pc, in_=src_pc)

    nc.compile()
    return nc


def single_d_out_region(out_d, b0):
    return out_d[b0 : b0 + NB_MEGA, :, 0:F_SINGLE]


_NC_CACHE = None


def _get_nc():
    global _NC_CACHE
    if _NC_CACHE is None:
        _NC_CACHE = _build_module()
    return _NC_CACHE


def run_sharded(single, pairwise, trace=False, **run_kwargs):
    """Run the SPMD kernel on 8 cores; returns (full_output, BassKernelResults)."""
    from concourse.bass_utils import run_bass_kernel_spmd

    nc = _get_nc()
    mpa, mpb = _constants()
    single = np.ascontiguousarray(single, dtype=np.float32)
    pairwise = np.ascontiguousarray(pairwise, dtype=np.float32)
    in_maps = []
    for c in range(N_CORES):
        sl = slice(c * B_LOCAL, (c + 1) * B_LOCAL)
        in_maps.append(
            {
                "single": single[sl],
                "pairwise": pairwise[sl],
                "mask_pair_a": mpa,
                "mask_pair_b": mpb,
            }
        )
    res = run_bass_kernel_spmd(
        nc, in_maps, core_ids=list(range(N_CORES)), trace=trace, **run_kwargs
    )
    out = np.concatenate([res.results[c]["out"] for c in range(N_CORES)], axis=0)
    return out, res


def kernel(single, pairwise):
    out, _ = run_sharded(single, pairwise)
    return out


# revision 21
# speedup vs baseline: 1.0578x; 1.0578x over previous
"""Trainium2 Bass kernel for nn_Mixer_81733227643296 (segment_reduce).

Computes, for single (B,16,256) and pairwise (B,256,64):
  out = concat([single, mean_up_bcast, mean_down_bcast, pair_up, pair_down], axis=2)
with out shape (B, 16, 896), B = 2048.

Sharding: pure data parallel over the batch across 8 NeuronCores (256 each).

Per-core structure, two independent pipelines:

1) single/means path (2 mega-tiles of 128 batch):
   - load single in batch-partition layout (fully linear DMA, 4KB packets)
   - spin sums via a 3-level DVE add tree along the free dim (exact fp32)
   - store singles straight from the tile to out[:, :, 0:256]
   - store means with a replication DMA: a 0-stride middle AP dim writes each
     batch row 16x to out[:, :, 256:768] (gpsimd/SWDGE supports 0-stride)

2) pairwise path (32 tiles of 8 batch):
   - pairwise tile partition p holds rows (2p, 2p+1) -> 512B DMA runs
   - pair segment sums on TensorE: combined up/down mask (M=32) fp32 matmuls
     accumulated over 2 K-chunks; psum layout (spin*16+elec, batch*64+f)
   - psum -> SBUF copy, then per-spin store DMA to out[:, :, 768:896]

DMA queue spread: loads on the SP HWDGE ring, stores on the ACT HWDGE ring,
means replication on gpsimd SWDGE.
"""

import numpy as np

N_EL = 16
N_UP = 8
N_DOWN = N_EL - N_UP
N_PAIR = N_EL * N_EL  # 240 off-diagonal pairs + 16 self entries
B_FULL = 2048
N_CORES = 8
B_LOCAL = B_FULL // N_CORES  # 256
F_SINGLE = 256
F_PAIR = 64
F_OUT = 3 * F_SINGLE + 2 * F_PAIR  # 896

NB_ITER = 8                    # batch items per pairwise iteration
N_ITERS = B_LOCAL // NB_ITER   # 32
NB_MEGA = 128                  # batch items per single/means mega-tile
N_MEGA = B_LOCAL // NB_MEGA    # 2


def _pairwise_masks(n_el, n_up):
    # identical to the reference construction
    eye = ~np.eye(n_el, dtype=bool)
    ups = np.zeros(n_el, dtype=bool)
    ups[:n_up] = True
    downs = ~ups
    up_rows, down_rows = [], []
    for e in range(n_el):
        m = np.zeros((n_el, n_el), dtype=bool)
        m[e, :] = ups
        mu = m[eye].reshape(-1)
        eu = np.zeros(n_el, dtype=bool)
        eu[e] = e < n_up
        up_rows.append(np.concatenate([mu, eu]))
        m = np.zeros((n_el, n_el), dtype=bool)
        m[e, :] = downs
        md = m[eye].reshape(-1)
        ed = np.zeros(n_el, dtype=bool)
        ed[e] = e >= n_up
        down_rows.append(np.concatenate([md, ed]))
    return np.stack(up_rows), np.stack(down_rows)  # each (n_el, n_el^2)


def _constants():
    mu, md = _pairwise_masks(N_EL, N_UP)
    mc = np.concatenate([mu, md], axis=0).astype(np.float32)  # (32, 256)
    # partition p of the pairwise tile holds rows 2p and 2p+1
    lhsT_pair_a = (mc[:, 0::2] / N_UP).T.copy()   # (128, 32), chunk rows 2p
    lhsT_pair_b = (mc[:, 1::2] / N_UP).T.copy()   # (128, 32), chunk rows 2p+1
    return lhsT_pair_a, lhsT_pair_b


def _build_module():
    import concourse.bass as bass
    import concourse.mybir as mybir
    import concourse.tile as tile
    from concourse import bacc

    f32 = mybir.dt.float32
    nc = bacc.Bacc("TRN2", target_bir_lowering=False, debug=False)

    single_d = nc.dram_tensor(
        "single", [B_LOCAL, N_EL, F_SINGLE], f32, kind="ExternalInput"
    ).ap()
    pair_d = nc.dram_tensor(
        "pairwise", [B_LOCAL, N_PAIR, F_PAIR], f32, kind="ExternalInput"
    ).ap()
    mpa_d = nc.dram_tensor("mask_pair_a", [128, 32], f32, kind="ExternalInput").ap()
    mpb_d = nc.dram_tensor("mask_pair_b", [128, 32], f32, kind="ExternalInput").ap()
    out_d = nc.dram_tensor(
        "out", [B_LOCAL, N_EL, F_OUT], f32, kind="ExternalOutput"
    ).ap()

    with tile.TileContext(nc) as tc:
        with (
            tc.tile_pool(name="consts", bufs=1) as cpool,
            tc.tile_pool(name="sgl", bufs=2) as sgl_pool,
            tc.tile_pool(name="tree", bufs=2) as tree_pool,
            tc.tile_pool(name="sums", bufs=2) as sums_pool,
            tc.tile_pool(name="pw", bufs=16) as pw_pool,
            tc.tile_pool(name="pairc", bufs=8) as pairc_pool,
            tc.tile_pool(name="psum_pair", bufs=8, space="PSUM") as psum_pair_pool,
        ):
            mpa = cpool.tile([128, 32], f32)
            nc.sync.dma_start(out=mpa[:], in_=mpa_d[:])
            mpb = cpool.tile([128, 32], f32)
            nc.sync.dma_start(out=mpb[:], in_=mpb_d[:])

            # ---- single/means path: mega loads up front; the two big
            # singles stores and two means stores are spaced through the pw
            # stream (pw prefetch depth rides through each store burst) ----
            sbs, sums_t = [], []
            for h in range(N_MEGA):
                b0 = h * NB_MEGA
                sb = sgl_pool.tile([128, N_EL * F_SINGLE], f32)
                nc.sync.dma_start(
                    out=sb[:],
                    in_=single_d[b0 : b0 + NB_MEGA].rearrange("b e f -> b (e f)"),
                )
                sbs.append(sb)

            def tree_path(h):
                sv = sbs[h][:].rearrange("b (e f) -> b e f", e=N_EL)
                tA = tree_pool.tile([128, 8 * F_SINGLE], f32)
                tAv = tA[:].rearrange("b (e f) -> b e f", e=8)
                nc.vector.tensor_add(
                    out=tAv[:, 0:4, :], in0=sv[:, 0:4, :], in1=sv[:, 4:8, :]
                )
                nc.vector.tensor_add(
                    out=tAv[:, 4:8, :], in0=sv[:, 8:12, :], in1=sv[:, 12:16, :]
                )
                tB = tree_pool.tile([128, 4 * F_SINGLE], f32)
                tBv = tB[:].rearrange("b (e f) -> b e f", e=4)
                nc.vector.tensor_add(
                    out=tBv[:, 0:2, :], in0=tAv[:, 0:2, :], in1=tAv[:, 2:4, :]
                )
                nc.vector.tensor_add(
                    out=tBv[:, 2:4, :], in0=tAv[:, 4:6, :], in1=tAv[:, 6:8, :]
                )
                sums = sums_pool.tile([128, 2 * F_SINGLE], f32)
                nc.vector.tensor_add(
                    out=sums[:, 0:F_SINGLE], in0=tBv[:, 0, :], in1=tBv[:, 1, :]
                )
                nc.vector.tensor_add(
                    out=sums[:, F_SINGLE:], in0=tBv[:, 2, :], in1=tBv[:, 3, :]
                )
                nc.scalar.mul(sums[:], sums[:], 1.0 / N_UP)
                sums_t.append(sums)

            def store_singles(h):
                b0 = h * NB_MEGA
                sv = sbs[h][:].rearrange("b (e f) -> b e f", e=N_EL)
                nc.gpsimd.dma_start(
                    out=out_d[b0 : b0 + NB_MEGA, :, 0:F_SINGLE], in_=sv
                )

            def store_means(h):
                b0 = h * NB_MEGA
                sums = sums_t[h]
                rep_src = bass.AP(
                    tensor=sums[:].tensor,
                    offset=sums[:].offset,
                    ap=[sums[:].ap[0], [0, N_EL], sums[:].ap[1]],
                )
                nc.gpsimd.dma_start(
                    out=out_d[b0 : b0 + NB_MEGA, :, F_SINGLE : 3 * F_SINGLE],
                    in_=rep_src,
                )

            # ---- pairwise path: 32 tiles of 8 batch, stores interleaved ----
            for it in range(N_ITERS):
                if it == 8:
                    store_singles(0)
                if it == 12:
                    tree_path(0)
                    store_means(0)
                if it == 20:
                    store_singles(1)
                if it == 24:
                    tree_path(1)
                    store_means(1)
                b0 = it * NB_ITER
                pw = pw_pool.tile([128, NB_ITER * 2 * F_PAIR], f32)
                pwv = pw[:].rearrange("p (b c f) -> p b c f", b=NB_ITER, c=2)

                src_pair = pair_d[b0 : b0 + NB_ITER].rearrange(
                    "b (p c) f -> p b (c f)", p=128
                )
                dst_pair = pw[:].rearrange("p (b cf) -> p b cf", b=NB_ITER)
                nc.sync.dma_start(out=dst_pair, in_=src_pair)

                ppair = psum_pair_pool.tile([32, NB_ITER * F_PAIR], f32)
                nc.tensor.matmul(
                    ppair[:], mpa[:], pwv[:, :, 0, :], start=True, stop=False
                )
                nc.tensor.matmul(
                    ppair[:], mpb[:], pwv[:, :, 1, :], start=False, stop=True
                )

                # pairc free size padded 512->576 so the per-spin store AP
                # can't greedily merge (i,b) into an illegal partition step
                pairc = pairc_pool.tile([32, NB_ITER * F_PAIR + F_PAIR], f32)
                nc.vector.tensor_copy(
                    out=pairc[:, 0 : NB_ITER * F_PAIR], in_=ppair[:]
                )

                # pair features: per-spin store, loop order (i, b, f)
                for s in range(2):
                    c0 = 3 * F_SINGLE + s * F_PAIR
                    src_pc = pairc[
                        s * N_EL : (s + 1) * N_EL, 0 : NB_ITER * F_PAIR
                    ].rearrange("i (b f) -> i b f", b=NB_ITER)
                    dst_pc = out_d[b0 : b0 + NB_ITER, :, c0 : c0 + F_PAIR].rearrange(
                        "b e f -> e b f"
                    )
                    nc.scalar.dma_start(out=dst_pc, in_=src_pc)

    nc.compile()
    return nc


def single_d_out_region(out_d, b0):
    return out_d[b0 : b0 + NB_MEGA, :, 0:F_SINGLE]


_NC_CACHE = None


def _get_nc():
    global _NC_CACHE
    if _NC_CACHE is None:
        _NC_CACHE = _build_module()
    return _NC_CACHE


def run_sharded(single, pairwise, trace=False, **run_kwargs):
    """Run the SPMD kernel on 8 cores; returns (full_output, BassKernelResults)."""
    from concourse.bass_utils import run_bass_kernel_spmd

    nc = _get_nc()
    mpa, mpb = _constants()
    single = np.ascontiguousarray(single, dtype=np.float32)
    pairwise = np.ascontiguousarray(pairwise, dtype=np.float32)
    in_maps = []
    for c in range(N_CORES):
        sl = slice(c * B_LOCAL, (c + 1) * B_LOCAL)
        in_maps.append(
            {
                "single": single[sl],
                "pairwise": pairwise[sl],
                "mask_pair_a": mpa,
                "mask_pair_b": mpb,
            }
        )
    res = run_bass_kernel_spmd(
        nc, in_maps, core_ids=list(range(N_CORES)), trace=trace, **run_kwargs
    )
    out = np.concatenate([res.results[c]["out"] for c in range(N_CORES)], axis=0)
    return out, res


def kernel(single, pairwise):
    out, _ = run_sharded(single, pairwise)
    return out
